# revision 1
# baseline (speedup 1.0000x reference)
"""Trainium2 Bass kernel for nn_BDH_6313601925221 (sparse_attention).

Model (reference.py):
  x = LN(embed[idx])                                   (B=1, T=1024, D=256)
  repeat 6 layers (shared weights):
    x_sparse = relu(einsum('btd,hdn->bhtn', x, encoder))   N=8192, NH=4
    QR       = rope(x_sparse)                              interleaved-pair rotation
    scores   = einsum('bhtn,bhsn->bhts', QR, QR) * strict_causal
    yKV      = LN(einsum('bhts,bsd->bhtd', scores, x))
    y_sparse = relu(einsum('bhtd,hdn->bhtn', yKV, encoder_v))
    yMLP     = (x_sparse*y_sparse).transpose -> (T, NH*N) @ decoder
    x        = LN(x + LN(yMLP))
  logits = x @ lm_head

Distribution (8 cores): core c = (head h=c//2, latent-half eta=c%2).
Each core computes the encoder/rope/scores path over its 4096 latent dims
(pairwise AllReduce of partial scores within the head pair), duplicates the
small yKV path, then computes y_sparse/xy/decoder over its latent half for
all tokens; one 8-rank AllReduce of the yMLP partials per layer.

Layouts: latent dim N is host-permuted so rope pairs are de-interleaved:
local tile 2j = even pair members, 2j+1 = odd. Inner products over N and
the decoder contraction are invariant to this permutation (weights are
permuted to match).

PSUM budget (8 banks): acc_a/acc_b/acc_c [128,1024] f32 (2 banks each,
bufs=1) carry all long-lived accumulations (score strips, yKV, yMLP);
ps_w [128,512] (bufs=2) carries transient matmul outputs.
"""

import math
import sys

import numpy as np

for _p in ("/opt/trn_rl_repo",):
    if _p not in sys.path:
        sys.path.insert(0, _p)

import concourse.bass as bass
import concourse.mybir as mybir
import concourse.tile as tile
from concourse import bacc
from concourse import bass_utils

# ---------------------------------------------------------------- constants
D = 256
NH = 4
N = 8192
T = 1024
N_LAYER = 6
VOCAB = 256
THETA = 2 ** 16
EPS = 1e-5
NCORES = 8

NHALF = N // 2          # 4096 latent dims per core
NPAIR = NHALF // 2      # 2048 rope pairs per core
NT = NHALF // 128       # 32 local n-tiles of 128
NJ = NT // 2            # 16 pair-blocks (tile 2j = evens, 2j+1 = odds)
TB = T // 128           # 8 token blocks
DC = D // 128           # 2 d-chunks

F16 = mybir.dt.float16
F32 = mybir.dt.float32
I32 = mybir.dt.int32
AX = mybir.AxisListType
ALU = mybir.AluOpType
ACTF = mybir.ActivationFunctionType

# kb -> (group, acc tag, column offset inside the [128,1024] acc tile)
SC_LAYOUT = {
    0: (0, "acc_a", 0),
    1: (0, "acc_b", 0),
    2: (0, "acc_c", 0),
    3: (1, "acc_a", 0),
    4: (1, "acc_b", 0),
    5: (1, "acc_b", 512),
    6: (1, "acc_c", 0),
    7: (1, "acc_c", 512),
}


def _bi(kb, qb):
    """Linear index of score block (kb, qb), kb <= qb."""
    return kb * TB - (kb * (kb - 1)) // 2 + (qb - kb)


def _ln_free(nc, pool, x_ap, eps_ap, out_f32=None, out_f16=None,
             skip_mean=False, n=None, name=""):
    """LayerNorm along the free dim of a [128, n] tile (per-partition stats)."""
    n = n if n is not None else x_ap.shape[-1]
    inv_n = 1.0 / n
    sq = pool.tile([128, n], F32, name=f"lnsq{name}", tag="lnsq")
    ssq = pool.tile([128, 1], F32, name=f"lnssq{name}", tag="lnssq")
    std = pool.tile([128, 1], F32, name=f"lnstd{name}", tag="lnstd")
    inv = pool.tile([128, 1], F32, name=f"lninv{name}", tag="lninv")
    if skip_mean:
        xm = x_ap
    else:
        mu = pool.tile([128, 1], F32, name=f"lnmu{name}", tag="lnmu")
        xm_t = pool.tile([128, n], F32, name=f"lnxm{name}", tag="lnxm")
        nc.vector.tensor_reduce(mu[:], x_ap, axis=AX.X, op=ALU.add)
        nc.scalar.mul(mu[:], mu[:], inv_n)
        nc.vector.tensor_scalar_sub(xm_t[:], x_ap, mu[:])
        xm = xm_t[:]
    nc.scalar.activation(sq[:], xm, ACTF.Square, accum_out=ssq[:])
    nc.scalar.activation(std[:], ssq[:], ACTF.Sqrt, bias=eps_ap, scale=inv_n)
    nc.vector.reciprocal(inv[:], std[:])
    if out_f32 is not None:
        nc.vector.tensor_scalar_mul(out_f32, xm, inv[:])
    if out_f16 is not None:
        nc.scalar.activation(out_f16, xm, ACTF.Copy, scale=inv[:])
    return xm, inv


def build_program(dbg=False, n_layer=N_LAYER, sim_single=False,
                  stub_sc_ar=False, stub_ym_ar=False, tiny_ar=False):
    if sim_single:
        stub_sc_ar = stub_ym_ar = True
    nc = bacc.Bacc("TRN2", target_bir_lowering=False, debug=False,
                   num_devices=NCORES)
    dbg_o = {}
    if dbg:
        dbg_o["x0"] = nc.dram_tensor("dbg_x0", [T, D], F32, kind="ExternalOutput")
        dbg_o["xs"] = nc.dram_tensor("dbg_xs", [256, T], F32, kind="ExternalOutput")
        dbg_o["qr"] = nc.dram_tensor("dbg_qr", [256, T], F32, kind="ExternalOutput")
        dbg_o["st"] = nc.dram_tensor("dbg_st", [36 * 128, 128], F32, kind="ExternalOutput")
        dbg_o["ykv"] = nc.dram_tensor("dbg_ykv", [T, D], F32, kind="ExternalOutput")
        dbg_o["ym"] = nc.dram_tensor("dbg_ym", [T, D], F32, kind="ExternalOutput")
        dbg_o["x1"] = nc.dram_tensor("dbg_x1", [T, D], F32, kind="ExternalOutput")
        dbg_o["ymp"] = nc.dram_tensor("dbg_ymp", [D, T], F16, kind="ExternalOutput")
        dbg_o["ykvT"] = nc.dram_tensor("dbg_ykvT", [256, T], F32, kind="ExternalOutput")

    # ------------------------------------------------------------- I/O decl
    idx_i = nc.dram_tensor("idx32", [T, 1], F32, kind="ExternalInput")
    embed_i = nc.dram_tensor("embed", [VOCAB, D], F32, kind="ExternalInput")
    enc_i = nc.dram_tensor("enc_sh", [D, NHALF], F16, kind="ExternalInput")
    encv_i = nc.dram_tensor("encv_sh", [D, NHALF], F16, kind="ExternalInput")
    dec_i = nc.dram_tensor("dec_sh", [NHALF, D], F16, kind="ExternalInput")
    lmh_i = nc.dram_tensor("lmh", [D, VOCAB], F16, kind="ExternalInput")
    cos_i = nc.dram_tensor("cos_sh", [NPAIR, T], F16, kind="ExternalInput")
    sin_i = nc.dram_tensor("sin_sh", [NPAIR, T], F16, kind="ExternalInput")
    cmask_i = nc.dram_tensor("cmask", [128, 128], F16, kind="ExternalInput")
    ident_i = nc.dram_tensor("ident", [128, 128], F16, kind="ExternalInput")
    ident32_i = nc.dram_tensor("ident32", [128, 128], F32, kind="ExternalInput")
    out_o = nc.dram_tensor("logits", [T, VOCAB], F32, kind="ExternalOutput")

    pair_groups = [[2 * h, 2 * h + 1] for h in range(NH)]
    all_group = [list(range(NCORES))]

    with tile.TileContext(nc) as tc:
      with (
        tc.tile_pool(name="persist", bufs=1) as pp,
        tc.tile_pool(name="work", bufs=2) as wp,
        tc.tile_pool(name="psW", bufs=2, space="PSUM") as psW,
        tc.tile_pool(name="psAcc", bufs=1, space="PSUM") as psAcc,
        tc.tile_pool(name="dram", bufs=1, space="DRAM") as dp,
      ):
        # ------------------------------------------------- persistent SBUF
        enc_sb = [pp.tile([128, NHALF], F16, name=f"enc{d}", tag=f"enc{d}")
                  for d in range(DC)]
        encv_sb = [pp.tile([128, NHALF], F16, name=f"encv{d}", tag=f"encv{d}")
                   for d in range(DC)]
        QR = [pp.tile([128, T], F16, name=f"qr{i}", tag=f"qr{i}")
              for i in range(NT)]
        ST = [pp.tile([128, 128], F16, name=f"st{i}", tag=f"st{i}")
              for i in range(36)]  # S^T blocks (kb,qb) kb<=qb, fp16, masked
        x_t32 = [pp.tile([128, D], F32, name=f"xt32_{i}", tag=f"xt32_{i}")
                 for i in range(TB)]
        x_t16 = [pp.tile([128, D], F16, name=f"xt16_{i}", tag=f"xt16_{i}")
                 for i in range(TB)]
        x_d16 = [pp.tile([128, T], F16, name=f"xd16_{i}", tag=f"xd16_{i}")
                 for i in range(DC)]
        ykv_t = [pp.tile([128, D], F16, name=f"ykvt{i}", tag=f"ykvt{i}")
                 for i in range(TB)]
        ykvT = [pp.tile([128, T], F16, name=f"ykvT{i}", tag=f"ykvT{i}")
                for i in range(DC)]
        cmask = pp.tile([128, 128], F16, name="cmaskt", tag="cmaskt")
        eps_t = pp.tile([128, 1], F32, name="eps_t", tag="eps_t")
        ident = pp.tile([128, 128], F16, name="identt", tag="identt")
        ident32 = pp.tile([128, 128], F32, name="identt32", tag="identt32")
        lmh_sb = [pp.tile([128, VOCAB], F16, name=f"lmh{d}", tag=f"lmh{d}")
                  for d in range(DC)]

        # ---------------------------------------------------- DRAM buffers
        xs_spill = dp.tile([NHALF, T], F16, name="xs_spill")
        sc_in0 = dp.tile([21 * 128, 128], F16, name="sc_in0")
        sc_out0 = dp.tile([21 * 128, 128], F16, name="sc_out0")
        sc_in1 = dp.tile([15 * 128, 128], F16, name="sc_in1")
        sc_out1 = dp.tile([15 * 128, 128], F16, name="sc_out1")
        tin = dp.tile([128, 128], F16, name="tin")
        touts = [dp.tile([128, 128], F16, name=f"tout{l}", tag=f"tout{l}")
                 for l in range(n_layer)]
        touts8 = [dp.tile([128, 128], F16, name=f"tout8{l}", tag=f"tout8{l}",
                  addr_space="Shared") for l in range(n_layer)]
        ym_in = dp.tile([D, T], F16, name="ym_in")
        ym_outs = [dp.tile([D, T], F16, name=f"ym_out{l}", tag=f"ym_out{l}",
                           addr_space="Shared") for l in range(n_layer)]

        def psw(name, shape=(128, 512), dtype=F32):
            return psW.tile(list(shape), dtype, name=name, tag="ps_w",
                            padded_shape=[128, 512])

        def dbg_dump16(dst_dram, row0, src_ap, w):
            tt = wp.tile([128, w], F32, name="dbgt", tag="dbgt", bufs=1)
            nc.vector.tensor_copy(tt[:], src_ap)
            nc.sync.dma_start(dst_dram[row0:row0 + 128, :], tt[:])

        # ------------------------------------------------------ load consts
        nc.gpsimd.memset(eps_t[:], EPS)
        nc.sync.dma_start(cmask[:], cmask_i[:, :])
        nc.sync.dma_start(ident[:], ident_i[:, :])
        nc.sync.dma_start(ident32[:], ident32_i[:, :])
        for d in range(DC):
            nc.sync.dma_start(enc_sb[d][:], enc_i[128 * d:128 * (d + 1), :])
            nc.sync.dma_start(encv_sb[d][:], encv_i[128 * d:128 * (d + 1), :])
            nc.sync.dma_start(lmh_sb[d][:], lmh_i[128 * d:128 * (d + 1), :])

        # ------------------------------------------------------- embedding
        # E_n = LN(embed) per vocab row; x0 = onehot(idx) @ E_n
        with tc.tile_pool(name="embed", bufs=1) as ep:
            E_n = [ep.tile([128, D], F16, name=f"en{v}", tag=f"en{v}")
                   for v in range(DC)]
            for v in range(DC):
                emb_raw = ep.tile([128, D], F32, name=f"emb_raw{v}",
                                  tag=f"emb_raw{v}")
                nc.sync.dma_start(emb_raw[:], embed_i[128 * v:128 * (v + 1), :])
                _ln_free(nc, wp, emb_raw[:], eps_t[:], out_f16=E_n[v][:],
                         name=f"emb{v}")

            iota_i32 = ep.tile([128, VOCAB], I32, name="iota_i32",
                               tag="iota_i32")
            nc.gpsimd.iota(iota_i32[:], pattern=[[1, VOCAB]], base=0,
                           channel_multiplier=0)
            iota_t = ep.tile([128, VOCAB], F32, name="iota_t", tag="iota_t")
            nc.vector.tensor_copy(iota_t[:], iota_i32[:])
            OHT = [ep.tile([128, T], F16, name=f"oht{v}", tag=f"oht{v}")
                   for v in range(DC)]
            for tb in range(TB):
                idx_col = wp.tile([128, 1], F32, name="idx_col", tag="idx_col")
                nc.sync.dma_start(idx_col[:], idx_i[128 * tb:128 * (tb + 1), :])
                oh_tm = wp.tile([128, VOCAB], F16, name="oh_tm", tag="oh_tm")
                nc.vector.tensor_scalar(oh_tm[:], iota_t[:], idx_col[:], None,
                                        op0=ALU.is_equal)
                for v in range(DC):
                    ps_t = psw(f"ps_tr_oh{tb}_{v}", (128, 128), F16)
                    nc.tensor.transpose(ps_t[:],
                                        oh_tm[:, 128 * v:128 * (v + 1)],
                                        ident[:])
                    nc.scalar.copy(OHT[v][:, 128 * tb:128 * (tb + 1)], ps_t[:])

            for tb in range(TB):
                ps_x = psw(f"ps_x0_{tb}", (128, D))
                for v in range(DC):
                    nc.tensor.matmul(ps_x[:],
                                     OHT[v][:, 128 * tb:128 * (tb + 1)],
                                     E_n[v][:], start=(v == 0),
                                     stop=(v == DC - 1))
                nc.vector.tensor_copy(x_t32[tb][:], ps_x[:])
                nc.scalar.copy(x_t16[tb][:], ps_x[:])
            for d in range(DC):
                for th in range(2):
                    ps_xd = psw(f"ps_xd_{d}_{th}")
                    for v in range(DC):
                        nc.tensor.matmul(
                            ps_xd[:], E_n[v][:, 128 * d:128 * (d + 1)],
                            OHT[v][:, 512 * th:512 * (th + 1)],
                            start=(v == 0), stop=(v == DC - 1))
                    nc.scalar.copy(x_d16[d][:, 512 * th:512 * (th + 1)],
                                   ps_xd[:])

        if dbg:
            for tb in range(TB):
                dbg_dump16(dbg_o["x0"], 128 * tb, x_t32[tb][:], D)

        # ============================================================ layers
        for layer in range(n_layer):
            # ---------------- phase 1a: x_sparse + rope + scores group 0
            acc = {t: psAcc.tile([128, 1024], F32, name=f"{t}_s0_{layer}",
                                 tag=t) for t in ("acc_a", "acc_b", "acc_c")}

            def sc_ap(kb, grp_acc):
                _, tag, off = SC_LAYOUT[kb]
                w = (TB - kb) * 128
                return grp_acc[tag][:, off:off + w]

            for j in range(NJ):
                ct = wp.tile([128, T], F16, name="cos_t", tag="cos_t")
                st_t = wp.tile([128, T], F16, name="sin_t", tag="sin_t")
                nc.sync.dma_start(ct[:], cos_i[128 * j:128 * (j + 1), :])
                nc.sync.dma_start(st_t[:], sin_i[128 * j:128 * (j + 1), :])
                xs_pair = []
                for par in range(2):  # even tile, odd tile
                    nt = 2 * j + par
                    xs_sb = wp.tile([128, T], F16, name="xs_sb", tag="xs_sb")
                    for th in range(2):
                        ps_e = psw(f"ps_enc_{layer}_{nt}_{th}")
                        for d in range(DC):
                            nc.tensor.matmul(
                                ps_e[:],
                                enc_sb[d][:, 128 * nt:128 * (nt + 1)],
                                x_d16[d][:, 512 * th:512 * (th + 1)],
                                start=(d == 0), stop=(d == DC - 1))
                        nc.scalar.activation(xs_sb[:, 512 * th:512 * (th + 1)],
                                             ps_e[:], ACTF.Relu)
                    nc.sync.dma_start(
                        xs_spill[128 * nt:128 * (nt + 1), :], xs_sb[:])
                    xs_pair.append(xs_sb)
                # rope: qr_e = xs_e*c - xs_o*s ; qr_o = xs_o*c + xs_e*s
                xe, xo = xs_pair[0], xs_pair[1]
                qe, qo = QR[2 * j], QR[2 * j + 1]
                p1 = wp.tile([128, T], F16, name="rp1", tag="rp1")
                p2 = wp.tile([128, T], F16, name="rp2", tag="rp2")
                nc.vector.tensor_mul(p1[:], xe[:], ct[:])
                nc.gpsimd.tensor_mul(p2[:], xo[:], st_t[:])
                nc.vector.tensor_sub(qe[:], p1[:], p2[:])
                nc.vector.tensor_mul(p1[:], xo[:], ct[:])
                nc.gpsimd.tensor_mul(p2[:], xe[:], st_t[:])
                nc.vector.tensor_add(qo[:], p1[:], p2[:])
                if dbg and layer == 0 and j == 0:
                    dbg_dump16(dbg_o["xs"], 0, xe[:], T)
                    dbg_dump16(dbg_o["xs"], 128, xo[:], T)
                    dbg_dump16(dbg_o["qr"], 0, qe[:], T)
                    dbg_dump16(dbg_o["qr"], 128, qo[:], T)
                # scores group-0 accumulation for these two n-chunks
                for par in range(2):
                    nt = 2 * j + par
                    first = (j == 0 and par == 0)
                    last = (j == NJ - 1 and par == 1)
                    for kb in range(TB):
                        if SC_LAYOUT[kb][0] != 0:
                            continue
                        dst = sc_ap(kb, acc)
                        w = (TB - kb) * 128
                        for nn in range(0, w, 512):
                            nw = min(512, w - nn)
                            nc.tensor.matmul(
                                dst[:, nn:nn + nw],
                                QR[nt][:, 128 * kb:128 * (kb + 1)],
                                QR[nt][:, 128 * kb + nn:128 * kb + nn + nw],
                                start=first, stop=last)
            # spill score group 0 to DRAM bounce (fp16 via SBUF)
            for kb in range(TB):
                if SC_LAYOUT[kb][0] != 0:
                    continue
                src = sc_ap(kb, acc)
                for qb in range(kb, TB):
                    s_sb = wp.tile([128, 128], F16, name="s_sb", tag="s_sb")
                    nc.scalar.copy(
                        s_sb[:],
                        src[:, 128 * (qb - kb):128 * (qb - kb + 1)])
                    nc.sync.dma_start(
                        sc_in0[128 * _bi(kb, qb):128 * (_bi(kb, qb) + 1), :],
                        s_sb[:])
            # AR of group 0 overlaps with the group-1 matmuls below
            if stub_sc_ar:
                nc.sync.dma_start(sc_out0[:, :], sc_in0[:, :])
                if tiny_ar:
                    nc.gpsimd.collective_compute(
                        "AllReduce", ALU.add, replica_groups=pair_groups,
                        ins=[tin.opt()], outs=[touts[layer].opt()])
            else:
                nc.gpsimd.collective_compute(
                    "AllReduce", ALU.add, replica_groups=pair_groups,
                    ins=[sc_in0.opt()], outs=[sc_out0.opt()])
            for kb in range(TB):
                if SC_LAYOUT[kb][0] != 0:
                    continue
                for qb in range(kb, TB):
                    blk = ST[_bi(kb, qb)]
                    nc.sync.dma_start(
                        blk[:],
                        sc_out0[128 * _bi(kb, qb):128 * (_bi(kb, qb) + 1), :])
                    if qb == kb:
                        nc.vector.tensor_mul(blk[:], blk[:], cmask[:])
            # ---------------- phase 1b: scores group 1 (QR resident)
            acc1 = {t: psAcc.tile([128, 1024], F32, name=f"{t}_s1_{layer}",
                                  tag=t) for t in ("acc_a", "acc_b", "acc_c")}
            for nt in range(NT):
                for kb in range(TB):
                    if SC_LAYOUT[kb][0] != 1:
                        continue
                    dst = sc_ap(kb, acc1)
                    w = (TB - kb) * 128
                    for nn in range(0, w, 512):
                        nw = min(512, w - nn)
                        nc.tensor.matmul(
                            dst[:, nn:nn + nw],
                            QR[nt][:, 128 * kb:128 * (kb + 1)],
                            QR[nt][:, 128 * kb + nn:128 * kb + nn + nw],
                            start=(nt == 0), stop=(nt == NT - 1))
            for kb in range(TB):
                if SC_LAYOUT[kb][0] != 1:
                    continue
                src = sc_ap(kb, acc1)
                for qb in range(kb, TB):
                    s_sb = wp.tile([128, 128], F16, name="s_sb", tag="s_sb")
                    nc.scalar.copy(
                        s_sb[:],
                        src[:, 128 * (qb - kb):128 * (qb - kb + 1)])
                    nc.sync.dma_start(
                        sc_in1[128 * (_bi(kb, qb) - 21):
                               128 * (_bi(kb, qb) - 20), :],
                        s_sb[:])
            # ---------------- scores AllReduce (group 1)
            if stub_sc_ar:
                nc.sync.dma_start(sc_out1[:, :], sc_in1[:, :])
            else:
                nc.gpsimd.collective_compute(
                    "AllReduce", ALU.add, replica_groups=pair_groups,
                    ins=[sc_in1.opt()], outs=[sc_out1.opt()])
            for kb in range(TB):
                if SC_LAYOUT[kb][0] != 1:
                    continue
                for qb in range(kb, TB):
                    blk = ST[_bi(kb, qb)]
                    nc.sync.dma_start(
                        blk[:],
                        sc_out1[128 * (_bi(kb, qb) - 21):
                                128 * (_bi(kb, qb) - 20), :])
                    if qb == kb:
                        nc.vector.tensor_mul(blk[:], blk[:], cmask[:])
            if dbg and layer == 0:
                for kb in range(TB):
                    for qb in range(kb, TB):
                        dbg_dump16(dbg_o["st"], 128 * _bi(kb, qb),
                                   ST[_bi(kb, qb)][:], 128)
            # ---------------- phase 2: yKV + LN + transpose
            # each of the 4 concurrent streams gets its own PSUM bank
            ykv_acc = {}
            for half in range(2):
                for ti, t in enumerate(("acc_a", "acc_b")):
                    ykv_acc[(half, ti)] = psAcc.tile(
                        [128, 1024], F32, name=f"{t}_ykv_{layer}_{half}",
                        tag=t)
            for qb in range(TB):
                ps_y = ykv_acc[(qb // 4, (qb % 4) // 2)][
                    :, 512 * (qb % 2):512 * (qb % 2) + D]
                for kb in range(qb + 1):
                    nc.tensor.matmul(ps_y, ST[_bi(kb, qb)][:], x_t16[kb][:],
                                     start=(kb == 0), stop=(kb == qb))
                _ln_free(nc, wp, ps_y, eps_t[:], out_f16=ykv_t[qb][:],
                         name=f"ykv{qb}")
                for d in range(DC):
                    ps_t = psw(f"ps_tr_ykv{qb}_{d}", (128, 128), F16)
                    nc.tensor.transpose(
                        ps_t[:], ykv_t[qb][:, 128 * d:128 * (d + 1)], ident[:])
                    nc.scalar.copy(ykvT[d][:, 128 * qb:128 * (qb + 1)],
                                   ps_t[:])
            if dbg and layer == 0:
                for qb in range(TB):
                    dbg_dump16(dbg_o["ykv"], 128 * qb, ykv_t[qb][:], D)
                for d in range(DC):
                    dbg_dump16(dbg_o["ykvT"], 128 * d, ykvT[d][:], T)
            # ---------------- phase 3: y_sparse, xy, decoder partials
            # yMLP^T partials [d, t]: one d-half per acc tile; the two
            # 512-wide t-chunks are separate streams in separate banks
            ym_acc = {}
            for half in range(2):
                t = ("acc_a", "acc_b")[half]
                ym_acc[half] = psAcc.tile([128, 1024], F32,
                                          name=f"{t}_ym_{layer}", tag=t)
            for nt in range(NT):
                dec_t = wp.tile([128, D], F16, name="dec_t", tag="dec_t")
                nc.sync.dma_start(dec_t[:], dec_i[128 * nt:128 * (nt + 1), :])
                xs_sb = wp.tile([128, T], F16, name="xs_sb2", tag="xs_sb2")
                nc.sync.dma_start(xs_sb[:],
                                  xs_spill[128 * nt:128 * (nt + 1), :])
                xy = wp.tile([128, T], F16, name="xy", tag="xy")
                for th in range(2):
                    ps_v = psw(f"ps_ysp_{layer}_{nt}_{th}")
                    for d in range(DC):
                        nc.tensor.matmul(
                            ps_v[:], encv_sb[d][:, 128 * nt:128 * (nt + 1)],
                            ykvT[d][:, 512 * th:512 * (th + 1)],
                            start=(d == 0), stop=(d == DC - 1))
                    # xy = relu(ys) * xs  (fused)
                    nc.vector.scalar_tensor_tensor(
                        xy[:, 512 * th:512 * (th + 1)], ps_v[:], 0.0,
                        xs_sb[:, 512 * th:512 * (th + 1)],
                        op0=ALU.max, op1=ALU.mult)
                for dh in range(DC):
                    for thc in range(2):
                        nc.tensor.matmul(
                            ym_acc[dh][:, 512 * thc:512 * (thc + 1)],
                            dec_t[:, 128 * dh:128 * (dh + 1)],
                            xy[:, 512 * thc:512 * (thc + 1)],
                            start=(nt == 0), stop=(nt == NT - 1))
            # ---------------- yMLP AllReduce (sum over heads & halves)
            for dh in range(DC):
                ym_sb = wp.tile([128, T], F16, name="ym_sb", tag="ym_sb",
                                bufs=1)
                nc.vector.tensor_copy(ym_sb[:], ym_acc[dh][:])
                nc.sync.dma_start(ym_in[128 * dh:128 * (dh + 1), :], ym_sb[:])
                if dbg and layer == 0:
                    nc.sync.dma_start(dbg_o["ymp"][128 * dh:128 * (dh + 1), :],
                                      ym_sb[:])
            ym_out = ym_outs[layer]
            if stub_ym_ar:
                nc.sync.dma_start(ym_out[:, :], ym_in[:, :])
                if tiny_ar:
                    nc.gpsimd.collective_compute(
                        "AllReduce", ALU.add, replica_groups=all_group,
                        ins=[tin.opt()], outs=[touts8[layer].opt()])
            else:
                nc.gpsimd.collective_compute(
                    "AllReduce", ALU.add, replica_groups=all_group,
                    ins=[ym_in.opt()], outs=[ym_out.opt()])
            # ---------------- tail: x = LN(x + LN(yMLP))
            um_d = [wp.tile([128, T], F16, name=f"um_d{dh}", tag=f"um_d{dh}",
                            bufs=1)
                    for dh in range(DC)]
            for dh in range(DC):
                nc.sync.dma_start(um_d[dh][:],
                                  ym_out[128 * dh:128 * (dh + 1), :])
            for tb in range(TB):
                u = wp.tile([128, D], F32, name="u_t", tag="u_t")
                for dh in range(DC):
                    ps_t16 = psw(f"ps_tru_{layer}_{tb}_{dh}", (128, 128), F16)
                    nc.tensor.transpose(
                        ps_t16[:], um_d[dh][:, 128 * tb:128 * (tb + 1)],
                        ident[:])
                    nc.scalar.copy(u[:, 128 * dh:128 * (dh + 1)], ps_t16[:])
                if dbg and layer == 0:
                    dbg_dump16(dbg_o["ym"], 128 * tb, u[:], D)
                xm_u, inv_u = _ln_free(nc, wp, u[:], eps_t[:], name=f"u{tb}")
                v = wp.tile([128, D], F32, name="v_t", tag="v_t")
                nc.vector.scalar_tensor_tensor(
                    v[:], xm_u, inv_u[:], x_t32[tb][:],
                    op0=ALU.mult, op1=ALU.add)
                _ln_free(nc, wp, v[:], eps_t[:], out_f32=x_t32[tb][:],
                         out_f16=x_t16[tb][:], skip_mean=True, name=f"v{tb}")
                if dbg and layer == 0:
                    dbg_dump16(dbg_o["x1"], 128 * tb, x_t32[tb][:], D)
                for d in range(DC):
                    ps_t = psw(f"ps_tr_x{layer}_{tb}_{d}", (128, 128), F16)
                    nc.tensor.transpose(
                        ps_t[:], x_t16[tb][:, 128 * d:128 * (d + 1)], ident[:])
                    nc.scalar.copy(x_d16[d][:, 128 * tb:128 * (tb + 1)],
                                   ps_t[:])

        # ------------------------------------------------------- lm head
        for tb in range(TB):
            ps_l = psw(f"ps_lg_{tb}", (128, VOCAB))
            for d in range(DC):
                nc.tensor.matmul(ps_l[:], x_d16[d][:, 128 * tb:128 * (tb + 1)],
                                 lmh_sb[d][:], start=(d == 0),
                                 stop=(d == DC - 1))
            lg_sb = wp.tile([128, VOCAB], F32, name="lg_sb", tag="lg_sb")
            nc.vector.tensor_copy(lg_sb[:], ps_l[:])
            nc.sync.dma_start(out_o[128 * tb:128 * (tb + 1), :], lg_sb[:])

    nc.compile()
    return nc


# ------------------------------------------------------------- host helpers
def _host_tables():
    """cos/sin rope tables in [pair, t] layout, mirroring reference fp32 math."""
    n = np.arange(N, dtype=np.float32)
    q = np.floor(n / 2.0) * 2.0
    freqs = (1.0 / (np.float32(THETA) ** (q / np.float32(N)))
             / np.float32(2.0 * math.pi)).astype(np.float32)
    t = np.arange(T, dtype=np.float32)
    phases = (t[:, None] * freqs[None, :]) % 1.0
    phases = phases * np.float32(2.0 * math.pi)
    cos = np.cos(phases).astype(np.float32)   # [T, N]
    sin = np.sin(phases).astype(np.float32)
    # pair p uses freq of n=2p; table[p, t]
    cos_p = cos[:, 0::2].T.copy()  # [N//2, T]
    sin_p = sin[:, 0::2].T.copy()
    return cos_p, sin_p


def _perm_local():
    """Local latent permutation: position -> (pair index, odd flag)."""
    pos_to_pair = np.empty(NHALF, dtype=np.int64)
    pos_is_odd = np.empty(NHALF, dtype=np.int64)
    for j in range(NJ):
        pr = np.arange(128) + 128 * j
        pos_to_pair[256 * j:256 * j + 128] = pr
        pos_is_odd[256 * j:256 * j + 128] = 0
        pos_to_pair[256 * j + 128:256 * j + 256] = pr
        pos_is_odd[256 * j + 128:256 * j + 256] = 1
    return pos_to_pair, pos_is_odd


_NC_CACHE = {}


def _get_nc():
    if "nc" not in _NC_CACHE:
        _NC_CACHE["nc"] = build_program()
    return _NC_CACHE["nc"]


def prepare_in_maps(idx, embed, encoder, encoder_v, decoder, lm_head):
    idx = np.asarray(idx)
    embed = np.asarray(embed, dtype=np.float32)
    encoder = np.asarray(encoder, dtype=np.float32)
    encoder_v = np.asarray(encoder_v, dtype=np.float32)
    decoder = np.asarray(decoder, dtype=np.float32)
    lm_head = np.asarray(lm_head, dtype=np.float32)

    cos_p, sin_p = _host_tables()
    pos_to_pair, pos_is_odd = _perm_local()

    cmask = (np.arange(128)[:, None] < np.arange(128)[None, :]).astype(np.float16)
    ident = np.eye(128, dtype=np.float16)
    ident32 = np.eye(128, dtype=np.float32)
    idx32 = idx.reshape(T).astype(np.float32).reshape(T, 1)
    lmh16 = lm_head.astype(np.float16)

    in_maps = []
    for c in range(NCORES):
        h, eta = c // 2, c % 2
        pair_g = NPAIR * eta + pos_to_pair          # global pair index
        n_orig = 2 * pair_g + pos_is_odd            # original n within head
        enc_sh = encoder[h][:, n_orig].astype(np.float16)
        encv_sh = encoder_v[h][:, n_orig].astype(np.float16)
        dec_sh = decoder[h * N + n_orig, :].astype(np.float16)
        cos_sh = cos_p[NPAIR * eta:NPAIR * (eta + 1), :].astype(np.float16)
        sin_sh = sin_p[NPAIR * eta:NPAIR * (eta + 1), :].astype(np.float16)
        in_maps.append({
            "idx32": idx32, "embed": embed, "enc_sh": enc_sh,
            "encv_sh": encv_sh, "dec_sh": dec_sh, "lmh": lmh16,
            "cos_sh": cos_sh, "sin_sh": sin_sh, "cmask": cmask,
            "ident": ident, "ident32": ident32,
        })
    return in_maps


def kernel(idx, embed, encoder, encoder_v, decoder, lm_head):
    in_maps = prepare_in_maps(idx, embed, encoder, encoder_v, decoder,
                              lm_head)
    nc = _get_nc()
    res = bass_utils.run_bass_kernel_spmd(nc, in_maps,
                                          core_ids=list(range(NCORES)))
    _NC_CACHE["last_results"] = res
    logits = np.asarray(res.results[0]["logits"], dtype=np.float32)
    return logits.reshape(1, T, VOCAB)



# revision 11
# speedup vs baseline: 40.5497x; 40.5497x over previous
"""Trainium2 Bass kernel for nn_BDH_6313601925221 (sparse_attention).

Model (reference.py):
  x = LN(embed[idx])                                   (B=1, T=1024, D=256)
  repeat 6 layers (shared weights):
    x_sparse = relu(einsum('btd,hdn->bhtn', x, encoder))   N=8192, NH=4
    QR       = rope(x_sparse)                              interleaved-pair rotation
    scores   = einsum('bhtn,bhsn->bhts', QR, QR) * strict_causal
    yKV      = LN(einsum('bhts,bsd->bhtd', scores, x))
    y_sparse = relu(einsum('bhtd,hdn->bhtn', yKV, encoder_v))
    yMLP     = (x_sparse*y_sparse).transpose -> (T, NH*N) @ decoder
    x        = LN(x + LN(yMLP))
  logits = x @ lm_head

Distribution (8 cores): core c = (head h=c//2, latent-half eta=c%2), 4096
latent dims each.  Scores feed only the linear yKV matmul, so each core
masks its PARTIAL scores and computes a partial yKV = S_part @ x; pair
AllReduces of yKV (split in two token halves, overlapped with remaining
score strips) replace any score exchange.  The decoder partials take two
8-rank AllReduces of yMLP^T (one per token half), overlapped with the
second phase-3 pass and the first tail half.

Scores run in fp8e4m3 DoubleRow perf mode (2 contraction rows/cycle):
QR is fp8-only, 16 tiles [128, 2T] (pair j's even rotation in cols 0:T,
odd in T:2T); a DoubleRow matmul contracts both halves via a [128, 2, w]
access pattern.  Rope cos/sin tables are fp8 SBUF-resident (loaded once,
layer-invariant).  QR carries a QSCALE prescale folded into the tables
and the fp16 score copy applies SSCALE; both cancel in the post-reduce
LayerNorm.  LayerNorms use bn_stats/bn_aggr.  encoder/encoder_v/decoder
shards and lm_head stay SBUF-resident; x_sparse spills to DRAM between
phase 1 and phase 3.
"""

import math
import sys

import numpy as np

for _p in ("/opt/trn_rl_repo",):
    if _p not in sys.path:
        sys.path.insert(0, _p)

import concourse.bass as bass
import concourse.mybir as mybir
import concourse.tile as tile
from concourse import bacc
from concourse import bass_utils

# ---------------------------------------------------------------- constants
D = 256
NH = 4
N = 8192
T = 1024
N_LAYER = 6
VOCAB = 256
THETA = 2 ** 16
EPS = 1e-5
NCORES = 8

NHALF = N // 2          # 4096 latent dims per core
NPAIR = NHALF // 2      # 2048 rope pairs per core
NT = NHALF // 128       # 32 local n-tiles of 128
NJ = NT // 2            # 16 pair-blocks (tile 2j = evens, 2j+1 = odds)
TB = T // 128           # 8 token blocks
DC = D // 128           # 2 d-chunks
QSCALE = 2.0            # QR prescale (folded into rope tables; LN cancels it)
SSCALE = 1.0 / 256.0    # score downscale on fp16 copy (keeps yKV in fp16 range)

F8 = mybir.dt.float8e4
F16 = mybir.dt.float16
F32 = mybir.dt.float32
I32 = mybir.dt.int32
AX = mybir.AxisListType
ALU = mybir.AluOpType
ACTF = mybir.ActivationFunctionType
PM_DR = mybir.MatmulPerfMode.DoubleRow


def _ln_bn(nc, pool, x_ap, eps_ap, out_f16=None, out_f32=None, name=""):
    """LayerNorm over the free dim via bn_stats/bn_aggr; returns (mv, inv)."""
    st6 = pool.tile([128, 6], F32, name=f"bnst{name}", tag="bnst")
    mv = pool.tile([128, 2], F32, name=f"bnmv{name}", tag="bnmv")
    std = pool.tile([128, 1], F32, name=f"bnstd{name}", tag="bnstd")
    inv = pool.tile([128, 1], F32, name=f"bninv{name}", tag="bninv")
    nc.vector.bn_stats(st6[:], x_ap)
    nc.vector.bn_aggr(mv[:], st6[:])
    nc.scalar.activation(std[:], mv[:, 1:2], ACTF.Sqrt, bias=eps_ap)
    nc.vector.reciprocal(inv[:], std[:])
    if out_f16 is not None:
        nc.vector.tensor_scalar(out_f16, x_ap, mv[:, 0:1], inv[:],
                                op0=ALU.subtract, op1=ALU.mult)
    if out_f32 is not None:
        nc.vector.tensor_scalar(out_f32, x_ap, mv[:, 0:1], inv[:],
                                op0=ALU.subtract, op1=ALU.mult)
    return mv, inv


def build_program(n_layer=N_LAYER, sim_single=False, tiny_ar=False, fp8=True,
                  fp8_tables=True, split_ar=True):
    nc = bacc.Bacc("TRN2", target_bir_lowering=False, debug=False,
                   num_devices=NCORES)

    # ------------------------------------------------------------- I/O decl
    idx_i = nc.dram_tensor("idx32", [T, 1], F32, kind="ExternalInput")
    embed_i = nc.dram_tensor("embed", [VOCAB, D], F32, kind="ExternalInput")
    enc_i = nc.dram_tensor("enc_sh", [D, NHALF], F16, kind="ExternalInput")
    encv_i = nc.dram_tensor("encv_sh", [D, NHALF], F16, kind="ExternalInput")
    dec_i = nc.dram_tensor("dec_sh", [NHALF, D], F16, kind="ExternalInput")
    lmh_i = nc.dram_tensor("lmh", [D, VOCAB], F16, kind="ExternalInput")
    TDT = F8 if fp8_tables else F16
    cos_i = nc.dram_tensor("cos_sh", [NPAIR, T], TDT, kind="ExternalInput")
    sin_i = nc.dram_tensor("sin_sh", [NPAIR, T], TDT, kind="ExternalInput")
    cmask_i = nc.dram_tensor("cmask", [128, 128], F16, kind="ExternalInput")
    ident_i = nc.dram_tensor("ident", [128, 128], F16, kind="ExternalInput")
    out_o = nc.dram_tensor("logits", [T, VOCAB], F32, kind="ExternalOutput")

    pair_groups = [[2 * h, 2 * h + 1] for h in range(NH)]
    all_group = [list(range(NCORES))]
    HT = T // 2  # token-half size

    with tile.TileContext(nc) as tc:
      with (
        tc.tile_pool(name="persist", bufs=1) as pp,
        tc.tile_pool(name="work", bufs=2) as wp,
        tc.tile_pool(name="psW", bufs=2, space="PSUM") as psW,
        tc.tile_pool(name="psStrip", bufs=1, space="PSUM") as psS,
        tc.tile_pool(name="psK", bufs=2, space="PSUM") as psK,
        tc.tile_pool(name="psM", bufs=1, space="PSUM") as psM,
        tc.tile_pool(name="dram", bufs=1, space="DRAM") as dp,
      ):
        # ------------------------------------------------- persistent SBUF
        enc_sb = [pp.tile([128, NHALF], F16, name=f"enc{d}", tag=f"enc{d}")
                  for d in range(DC)]
        encv_sb = [pp.tile([128, NHALF], F16, name=f"encv{d}", tag=f"encv{d}")
                   for d in range(DC)]
        dec_sb = [pp.tile([128, D], F16, name=f"dec{nt}", tag=f"dec{nt}")
                  for nt in range(NT)]
        cos_sb = pp.tile([128, NJ * T], TDT, name="cos_sb", tag="cos_sb")
        sin_sb = pp.tile([128, NJ * T], TDT, name="sin_sb", tag="sin_sb")
        if fp8:
            q8 = [pp.tile([128, 2 * T], F8, name=f"q8_{j}", tag=f"q8_{j}")
                  for j in range(NJ)]
        else:
            QR16 = [pp.tile([128, T], F16, name=f"qr{i}", tag=f"qr{i}")
                    for i in range(NT)]
        ST_str = [pp.tile([128, (TB - kb) * 128], F16, name=f"st{kb}",
                          tag=f"st{kb}") for kb in range(TB)]
        x_t32 = [pp.tile([128, D], F32, name=f"xt32_{i}", tag=f"xt32_{i}")
                 for i in range(TB)]
        x_t16 = [pp.tile([128, D], F16, name=f"xt16_{i}", tag=f"xt16_{i}")
                 for i in range(TB)]
        x_d16 = [pp.tile([128, T], F16, name=f"xd16_{i}", tag=f"xd16_{i}")
                 for i in range(DC)]
        ykv_t = [pp.tile([128, D], F16, name=f"ykvt{i}", tag=f"ykvt{i}")
                 for i in range(TB)]
        ykvT = pp.tile([128, DC * T], F16, name="ykvT", tag="ykvT")
        cmask = pp.tile([128, 128], F16, name="cmaskt", tag="cmaskt")
        eps_t = pp.tile([128, 1], F32, name="eps_t", tag="eps_t")
        ident = pp.tile([128, 128], F16, name="identt", tag="identt")
        lmh_sb = [pp.tile([128, VOCAB], F16, name=f"lmh{d}", tag=f"lmh{d}")
                  for d in range(DC)]

        # ---------------------------------------------------- DRAM buffers
        xs_spill = dp.tile([NHALF, T], F16, name="xs_spill")
        nhalves = 2 if split_ar else 1
        ykv_ins = [dp.tile([T // nhalves, D], F16, name=f"ykv_in{g}")
                   for g in range(nhalves)]
        ykv_outs = [[dp.tile([T // nhalves, D], F16, name=f"ykv_out{l}_{g}")
                     for g in range(nhalves)] for l in range(n_layer)]
        ym_ins = [dp.tile([D, T // nhalves], F16, name=f"ym_in{g}")
                  for g in range(nhalves)]
        ym_outs = [[dp.tile([D, T // nhalves], F16, name=f"ym_out{l}_{g}",
                            addr_space="Shared") for g in range(nhalves)]
                   for l in range(n_layer)]
        tin = dp.tile([128, 128], F16, name="tin")
        touts = [dp.tile([128, 128], F16, name=f"tout{l}", tag=f"tout{l}")
                 for l in range(n_layer)]
        touts8 = [dp.tile([128, 128], F16, name=f"tout8{l}", tag=f"tout8{l}",
                          addr_space="Shared") for l in range(n_layer)]

        def psw(name, shape=(128, 512), dtype=F32):
            return psW.tile(list(shape), dtype, name=name, tag="ps_w",
                            padded_shape=[128, 512])

        # ------------------------------------------------------ load consts
        nc.gpsimd.memset(eps_t[:], EPS)
        nc.sync.dma_start(cmask[:], cmask_i[:, :])
        nc.sync.dma_start(ident[:], ident_i[:, :])
        for d in range(DC):
            nc.sync.dma_start(enc_sb[d][:], enc_i[128 * d:128 * (d + 1), :])
            nc.sync.dma_start(encv_sb[d][:], encv_i[128 * d:128 * (d + 1), :])
            nc.sync.dma_start(lmh_sb[d][:], lmh_i[128 * d:128 * (d + 1), :])
        for nt in range(NT):
            nc.sync.dma_start(dec_sb[nt][:], dec_i[128 * nt:128 * (nt + 1), :])
        for j in range(NJ):
            nc.sync.dma_start(cos_sb[:, T * j:T * (j + 1)],
                              cos_i[128 * j:128 * (j + 1), :])
            nc.sync.dma_start(sin_sb[:, T * j:T * (j + 1)],
                              sin_i[128 * j:128 * (j + 1), :])

        # ------------------------------------------------------- embedding
        with tc.tile_pool(name="embed", bufs=1) as ep:
            E_n = [ep.tile([128, D], F16, name=f"en{v}", tag=f"en{v}")
                   for v in range(DC)]
            for v in range(DC):
                emb_raw = ep.tile([128, D], F32, name=f"emb_raw{v}",
                                  tag=f"emb_raw{v}")
                nc.sync.dma_start(emb_raw[:], embed_i[128 * v:128 * (v + 1), :])
                _ln_bn(nc, wp, emb_raw[:], eps_t[:], out_f16=E_n[v][:],
                       name=f"emb{v}")

            iota_i32 = ep.tile([128, VOCAB], I32, name="iota_i32",
                               tag="iota_i32")
            nc.gpsimd.iota(iota_i32[:], pattern=[[1, VOCAB]], base=0,
                           channel_multiplier=0)
            iota_t = ep.tile([128, VOCAB], F32, name="iota_t", tag="iota_t")
            nc.vector.tensor_copy(iota_t[:], iota_i32[:])
            OHT = [ep.tile([128, T], F16, name=f"oht{v}", tag=f"oht{v}")
                   for v in range(DC)]
            for tb in range(TB):
                idx_col = wp.tile([128, 1], F32, name="idx_col", tag="idx_col")
                nc.sync.dma_start(idx_col[:], idx_i[128 * tb:128 * (tb + 1), :])
                oh_tm = wp.tile([128, VOCAB], F16, name="oh_tm", tag="oh_tm")
                nc.vector.tensor_scalar(oh_tm[:], iota_t[:], idx_col[:], None,
                                        op0=ALU.is_equal)
                for v in range(DC):
                    ps_t = psw(f"ps_tr_oh{tb}_{v}", (128, 128), F16)
                    nc.tensor.transpose(ps_t[:],
                                        oh_tm[:, 128 * v:128 * (v + 1)],
                                        ident[:])
                    nc.scalar.copy(OHT[v][:, 128 * tb:128 * (tb + 1)], ps_t[:])

            for tb in range(TB):
                ps_x = psw(f"ps_x0_{tb}", (128, D))
                for v in range(DC):
                    nc.tensor.matmul(ps_x[:],
                                     OHT[v][:, 128 * tb:128 * (tb + 1)],
                                     E_n[v][:], start=(v == 0),
                                     stop=(v == DC - 1))
                nc.vector.tensor_copy(x_t32[tb][:], ps_x[:])
                nc.scalar.copy(x_t16[tb][:], ps_x[:])
            for d in range(DC):
                for th in range(2):
                    ps_xd = psw(f"ps_xd_{d}_{th}")
                    for v in range(DC):
                        nc.tensor.matmul(
                            ps_xd[:], E_n[v][:, 128 * d:128 * (d + 1)],
                            OHT[v][:, 512 * th:512 * (th + 1)],
                            start=(v == 0), stop=(v == DC - 1))
                    nc.scalar.copy(x_d16[d][:, 512 * th:512 * (th + 1)],
                                   ps_xd[:])

        # ============================================================ layers
        for layer in range(n_layer):
            # -------- phase 1: x_sparse (spill) + rope -> fp8 QR tiles
            for j in range(NJ):
                ct = cos_sb[:, T * j:T * (j + 1)]
                st_t = sin_sb[:, T * j:T * (j + 1)]
                xs_pair = []
                for par in range(2):  # even tile, odd tile
                    nt = 2 * j + par
                    xs_sb = wp.tile([128, T], F16, name="xs_sb", tag="xs_sb")
                    for th in range(2):
                        ps_e = psw(f"ps_enc_{layer}_{nt}_{th}")
                        for d in range(DC):
                            nc.tensor.matmul(
                                ps_e[:],
                                enc_sb[d][:, 128 * nt:128 * (nt + 1)],
                                x_d16[d][:, 512 * th:512 * (th + 1)],
                                start=(d == 0), stop=(d == DC - 1))
                        nc.scalar.activation(xs_sb[:, 512 * th:512 * (th + 1)],
                                             ps_e[:], ACTF.Relu)
                    nc.sync.dma_start(
                        xs_spill[128 * nt:128 * (nt + 1), :], xs_sb[:])
                    xs_pair.append(xs_sb)
                # rope: qr_e = xs_e*c - xs_o*s ; qr_o = xs_o*c + xs_e*s
                xe, xo = xs_pair[0], xs_pair[1]
                if fp8:
                    qe = q8[j][:, 0:T]
                    qo = q8[j][:, T:2 * T]
                else:
                    qe = QR16[2 * j][:]
                    qo = QR16[2 * j + 1][:]
                p1 = wp.tile([128, T], F16, name="rp1", tag="rp1")
                p2 = wp.tile([128, T], F16, name="rp2", tag="rp2")
                nc.vector.tensor_mul(p1[:], xe[:], ct)
                nc.gpsimd.tensor_mul(p2[:], xo[:], st_t)
                nc.vector.tensor_sub(qe, p1[:], p2[:])
                nc.vector.tensor_mul(p1[:], xo[:], ct)
                nc.gpsimd.tensor_mul(p2[:], xe[:], st_t)
                nc.vector.tensor_add(qo, p1[:], p2[:])

            # -------- phase 2: score strips (kb-major) + partial yKV
            def do_strip(kb):
                w = (TB - kb) * 128
                strip = psS.tile([128, 1024], F32, name=f"strip_{layer}_{kb}",
                                 tag="strip")
                for c0 in range(0, w, 512):
                    cw = min(512, w - c0)
                    if fp8:
                        for j in range(NJ):
                            qv = q8[j][:].rearrange("p (two f) -> p two f",
                                                    two=2)
                            nc.tensor.matmul(
                                strip[:, c0:c0 + cw],
                                qv[:, :, 128 * kb:128 * (kb + 1)],
                                qv[:, :, 128 * kb + c0:128 * kb + c0 + cw],
                                start=(j == 0), stop=(j == NJ - 1),
                                perf_mode=PM_DR)
                    else:
                        for ntq in range(NT):
                            nc.tensor.matmul(
                                strip[:, c0:c0 + cw],
                                QR16[ntq][:, 128 * kb:128 * (kb + 1)],
                                QR16[ntq][:, 128 * kb + c0:128 * kb + c0 + cw],
                                start=(ntq == 0), stop=(ntq == NT - 1))
                # scale+copy strip to fp16, strict-causal mask on diag block
                for c0 in range(0, w, 512):
                    cw = min(512, w - c0)
                    nc.scalar.mul(ST_str[kb][:, c0:c0 + cw],
                                  strip[:, c0:c0 + cw], SSCALE)
                nc.vector.tensor_mul(ST_str[kb][:, 0:128],
                                     ST_str[kb][:, 0:128], cmask[:])
                # partial yKV for query block qb == kb (strips kp<=kb done)
                ykv_ps = psK.tile([128, D], F32, name=f"ykv_{layer}_{kb}",
                                  tag="ykv", padded_shape=[128, 512])
                for kp in range(kb + 1):
                    nc.tensor.matmul(
                        ykv_ps[:],
                        ST_str[kp][:, 128 * (kb - kp):128 * (kb - kp + 1)],
                        x_t16[kp][:], start=(kp == 0), stop=(kp == kb))
                ykv_sb = wp.tile([128, D], F16, name="ykv_sb", tag="ykv_sb")
                nc.scalar.copy(ykv_sb[:], ykv_ps[:])
                g, r = (kb // 4, kb % 4) if split_ar else (0, kb)
                nc.sync.dma_start(ykv_ins[g][128 * r:128 * (r + 1), :],
                                  ykv_sb[:])

            def fire_ykv_ar(g):
                if sim_single:
                    nc.sync.dma_start(ykv_outs[layer][g][:, :],
                                      ykv_ins[g][:, :])
                    if tiny_ar and g == 0:
                        nc.gpsimd.collective_compute(
                            "AllReduce", ALU.add, replica_groups=pair_groups,
                            ins=[tin.opt()], outs=[touts[layer].opt()])
                else:
                    nc.gpsimd.collective_compute(
                        "AllReduce", ALU.add, replica_groups=pair_groups,
                        ins=[ykv_ins[g].opt()],
                        outs=[ykv_outs[layer][g].opt()])

            if split_ar:
                for kb in range(4):
                    do_strip(kb)
                fire_ykv_ar(0)
                for kb in range(4, TB):
                    do_strip(kb)
                fire_ykv_ar(1)
            else:
                for kb in range(TB):
                    do_strip(kb)
                fire_ykv_ar(0)

            # -------- phase Y: LN(yKV) + transpose to [d, t] (per half)
            def do_y(qb):
                g, r = (qb // 4, qb % 4) if split_ar else (0, qb)
                yk = wp.tile([128, D], F16, name="yk_ld", tag="yk_ld")
                nc.sync.dma_start(
                    yk[:], ykv_outs[layer][g][128 * r:128 * (r + 1), :])
                _ln_bn(nc, wp, yk[:], eps_t[:], out_f16=ykv_t[qb][:],
                       name=f"ykv{qb}")
                ps_t = psw(f"ps_tr_ykv{qb}", (128, D), F16)
                for d in range(DC):
                    nc.tensor.transpose(
                        ps_t[:, 128 * d:128 * (d + 1)],
                        ykv_t[qb][:, 128 * d:128 * (d + 1)], ident[:])
                for d in range(DC):
                    nc.scalar.copy(ykvT[:, T * d + 128 * qb:
                                        T * d + 128 * (qb + 1)],
                                   ps_t[:, 128 * d:128 * (d + 1)])

            # -------- phase 3: y_sparse, xy, decoder partials (col groups)
            def do_p3(gg):
                cg = 512 * gg
                ym_ps = [psM.tile([128, 512], F32,
                                  name=f"ym_{layer}_{gg}_{dh}",
                                  tag=f"ym{dh}") for dh in range(DC)]
                for nt in range(NT):
                    xs_sb = wp.tile([128, 512], F16, name="xs_sb2",
                                    tag="xs_sb2")
                    nc.sync.dma_start(
                        xs_sb[:],
                        xs_spill[128 * nt:128 * (nt + 1), cg:cg + 512])
                    ps_v = psw(f"ps_ysp_{layer}_{gg}_{nt}")
                    for d in range(DC):
                        nc.tensor.matmul(
                            ps_v[:], encv_sb[d][:, 128 * nt:128 * (nt + 1)],
                            ykvT[:, T * d + cg:T * d + cg + 512],
                            start=(d == 0), stop=(d == DC - 1))
                    xy = wp.tile([128, 512], F16, name="xy", tag="xy")
                    nc.vector.scalar_tensor_tensor(
                        xy[:], ps_v[:], 0.0, xs_sb[:],
                        op0=ALU.max, op1=ALU.mult)
                    for dh in range(DC):
                        nc.tensor.matmul(
                            ym_ps[dh][:],
                            dec_sb[nt][:, 128 * dh:128 * (dh + 1)],
                            xy[:], start=(nt == 0), stop=(nt == NT - 1))
                g = gg if split_ar else 0
                for dh in range(DC):
                    ym_sb = wp.tile([128, 512], F16, name="ym_sb",
                                    tag="ym_sb")
                    nc.vector.tensor_copy(ym_sb[:], ym_ps[dh][:])
                    if split_ar:
                        nc.sync.dma_start(
                            ym_ins[g][128 * dh:128 * (dh + 1), :], ym_sb[:])
                    else:
                        nc.sync.dma_start(
                            ym_ins[0][128 * dh:128 * (dh + 1),
                                      cg:cg + 512], ym_sb[:])

            def fire_ym_ar(g):
                if sim_single:
                    nc.sync.dma_start(ym_outs[layer][g][:, :],
                                      ym_ins[g][:, :])
                    if tiny_ar and g == 0:
                        nc.gpsimd.collective_compute(
                            "AllReduce", ALU.add, replica_groups=all_group,
                            ins=[tin.opt()], outs=[touts8[layer].opt()])
                else:
                    nc.gpsimd.collective_compute(
                        "AllReduce", ALU.add, replica_groups=all_group,
                        ins=[ym_ins[g].opt()],
                        outs=[ym_outs[layer][g].opt()])

            # -------- tail: x = LN(x + LN(yMLP)) for one token half
            def do_tail(half):
                g = half if split_ar else 0
                hw_ = HT if split_ar else T
                um_d = [wp.tile([128, hw_], F16, name=f"um_d{half}_{dh}",
                                tag=f"um_d{dh}", bufs=1) for dh in range(DC)]
                for dh in range(DC):
                    nc.sync.dma_start(
                        um_d[dh][:],
                        ym_outs[layer][g][128 * dh:128 * (dh + 1), :])
                tbs = range(4 * half, 4 * half + 4) if split_ar else range(TB)
                for tb in tbs:
                    r = tb - 4 * half if split_ar else tb
                    ps_tu = psw(f"ps_tru_{layer}_{tb}", (128, D), F16)
                    for dh in range(DC):
                        nc.tensor.transpose(
                            ps_tu[:, 128 * dh:128 * (dh + 1)],
                            um_d[dh][:, 128 * r:128 * (r + 1)], ident[:])
                    u = wp.tile([128, D], F32, name="u_t", tag="u_t")
                    nc.scalar.copy(u[:], ps_tu[:])
                    mv_u, inv_u = _ln_bn(nc, wp, u[:], eps_t[:],
                                         name=f"u{tb}")
                    lnu = wp.tile([128, D], F32, name="lnu_t", tag="lnu_t")
                    nc.vector.tensor_scalar(lnu[:], u[:], mv_u[:, 0:1],
                                            inv_u[:], op0=ALU.subtract,
                                            op1=ALU.mult)
                    v = wp.tile([128, D], F32, name="v_t", tag="v_t")
                    nc.gpsimd.tensor_add(v[:], lnu[:], x_t32[tb][:])
                    mv_v, inv_v = _ln_bn(nc, wp, v[:], eps_t[:],
                                         name=f"v{tb}")
                    nc.vector.tensor_scalar(x_t32[tb][:], v[:], mv_v[:, 0:1],
                                            inv_v[:], op0=ALU.subtract,
                                            op1=ALU.mult)
                    nc.scalar.activation(x_t16[tb][:], x_t32[tb][:],
                                         ACTF.Copy)
                    ps_tx = psw(f"ps_trx_{layer}_{tb}", (128, D), F16)
                    for d in range(DC):
                        nc.tensor.transpose(
                            ps_tx[:, 128 * d:128 * (d + 1)],
                            x_t16[tb][:, 128 * d:128 * (d + 1)], ident[:])
                    for d in range(DC):
                        nc.scalar.copy(x_d16[d][:, 128 * tb:128 * (tb + 1)],
                                       ps_tx[:, 128 * d:128 * (d + 1)])

            if split_ar:
                for qb in range(4):
                    do_y(qb)
                do_p3(0)
                for qb in range(4, TB):
                    do_y(qb)
                fire_ym_ar(0)   # after p3(0) dec accumulation done
                do_p3(1)
                fire_ym_ar(1)
                do_tail(0)
                do_tail(1)
            else:
                for qb in range(TB):
                    do_y(qb)
                do_p3(0)
                do_p3(1)
                fire_ym_ar(0)
                do_tail(0)

        # ------------------------------------------------------- lm head
        for tb in range(TB):
            ps_l = psw(f"ps_lg_{tb}", (128, VOCAB))
            for d in range(DC):
                nc.tensor.matmul(ps_l[:], x_d16[d][:, 128 * tb:128 * (tb + 1)],
                                 lmh_sb[d][:], start=(d == 0),
                                 stop=(d == DC - 1))
            lg_sb = wp.tile([128, VOCAB], F32, name="lg_sb", tag="lg_sb")
            nc.vector.tensor_copy(lg_sb[:], ps_l[:])
            nc.sync.dma_start(out_o[128 * tb:128 * (tb + 1), :], lg_sb[:])

    nc.compile()
    return nc


# ------------------------------------------------------------- host helpers
def _host_tables():
    """cos/sin rope tables in [pair, t] layout, mirroring reference fp32 math."""
    n = np.arange(N, dtype=np.float32)
    q = np.floor(n / 2.0) * 2.0
    freqs = (1.0 / (np.float32(THETA) ** (q / np.float32(N)))
             / np.float32(2.0 * math.pi)).astype(np.float32)
    t = np.arange(T, dtype=np.float32)
    phases = (t[:, None] * freqs[None, :]) % 1.0
    phases = phases * np.float32(2.0 * math.pi)
    cos = np.cos(phases).astype(np.float32)   # [T, N]
    sin = np.sin(phases).astype(np.float32)
    # pair p uses freq of n=2p; table[p, t]
    cos_p = cos[:, 0::2].T.copy()  # [N//2, T]
    sin_p = sin[:, 0::2].T.copy()
    return cos_p * QSCALE, sin_p * QSCALE


def _perm_local():
    """Local latent permutation: position -> (pair index, odd flag)."""
    pos_to_pair = np.empty(NHALF, dtype=np.int64)
    pos_is_odd = np.empty(NHALF, dtype=np.int64)
    for j in range(NJ):
        pr = np.arange(128) + 128 * j
        pos_to_pair[256 * j:256 * j + 128] = pr
        pos_is_odd[256 * j:256 * j + 128] = 0
        pos_to_pair[256 * j + 128:256 * j + 256] = pr
        pos_is_odd[256 * j + 128:256 * j + 256] = 1
    return pos_to_pair, pos_is_odd


_NC_CACHE = {}


def _get_nc():
    if "nc" not in _NC_CACHE:
        _NC_CACHE["nc"] = build_program()
    return _NC_CACHE["nc"]


def prepare_in_maps(idx, embed, encoder, encoder_v, decoder, lm_head,
                    fp8_tables=True):
    import ml_dtypes
    idx = np.asarray(idx)
    embed = np.asarray(embed, dtype=np.float32)
    encoder = np.asarray(encoder, dtype=np.float32)
    encoder_v = np.asarray(encoder_v, dtype=np.float32)
    decoder = np.asarray(decoder, dtype=np.float32)
    lm_head = np.asarray(lm_head, dtype=np.float32)

    cos_p, sin_p = _host_tables()
    pos_to_pair, pos_is_odd = _perm_local()

    cmask = (np.arange(128)[:, None] < np.arange(128)[None, :]).astype(np.float16)
    ident = np.eye(128, dtype=np.float16)
    idx32 = idx.reshape(T).astype(np.float32).reshape(T, 1)
    lmh16 = lm_head.astype(np.float16)

    def table(x):
        if fp8_tables:
            return x.astype(ml_dtypes.float8_e4m3fn).view(np.uint8)
        return x.astype(np.float16)

    in_maps = []
    for c in range(NCORES):
        h, eta = c // 2, c % 2
        pair_g = NPAIR * eta + pos_to_pair          # global pair index
        n_orig = 2 * pair_g + pos_is_odd            # original n within head
        enc_sh = encoder[h][:, n_orig].astype(np.float16)
        encv_sh = encoder_v[h][:, n_orig].astype(np.float16)
        dec_sh = decoder[h * N + n_orig, :].astype(np.float16)
        cos_sh = table(cos_p[NPAIR * eta:NPAIR * (eta + 1), :])
        sin_sh = table(sin_p[NPAIR * eta:NPAIR * (eta + 1), :])
        in_maps.append({
            "idx32": idx32, "embed": embed, "enc_sh": enc_sh,
            "encv_sh": encv_sh, "dec_sh": dec_sh, "lmh": lmh16,
            "cos_sh": cos_sh, "sin_sh": sin_sh, "cmask": cmask,
            "ident": ident,
        })
    return in_maps


def kernel(idx, embed, encoder, encoder_v, decoder, lm_head):
    in_maps = prepare_in_maps(idx, embed, encoder, encoder_v, decoder,
                              lm_head)
    nc = _get_nc()
    res = bass_utils.run_bass_kernel_spmd(nc, in_maps,
                                          core_ids=list(range(NCORES)))
    _NC_CACHE["last_results"] = res
    logits = np.asarray(res.results[0]["logits"], dtype=np.float32)
    return logits.reshape(1, T, VOCAB)
